# revision 1
# baseline (speedup 1.0000x reference)
"""Causal self-attention TRN2 Bass kernel (8 NeuronCores).

Sharding: core c handles batch b = c//4 and heads [4*(c%4), 4*(c%4)+4).
Each core computes its heads' QKV projection, causal attention, and the
partial output projection ctx_slice @ w_out_rows; the host sums the 4
partials per batch (exact, since the projection is linear over head
channels) and adds the constant bias terms.

Numerics: matmuls in float32r (TF32-like, ~13-bit mantissa, full PE rate
at N>=256); softmax logits in fp32 PSUM with exact row-max subtraction;
P and V in bf16 (linear error only).
"""
import math
import os

import numpy as np

import concourse.bacc as bacc
import concourse.bass as bass
import concourse.mybir as mybir
import concourse.tile as tile
from concourse.bass import ds, ts
from concourse.bass_utils import run_bass_kernel_spmd
from concourse.masks import make_identity

# problem shapes (hardcoded per contract)
B, T, C = 2, 2048, 1024
H, D = 16, 64
P = 128
CG = C // P            # 8 contraction tiles over channels
TT = T // P            # 16 token tiles of 128
NG = T // 512          # 4 q-groups of 512
HPAIRS = 2             # head-pairs per core (4 heads/core)
HC = 256               # head channels per core (4 heads * 64)
WLAST = [256, 256, 384, 512]   # ragged width of the diagonal k-tile per qt%4
NEG = -1.0e30

F32 = mybir.dt.float32
F32R = mybir.dt.float32r
BF16 = mybir.dt.bfloat16
AX = mybir.AxisListType
OP = mybir.AluOpType
ACTF = mybir.ActivationFunctionType

_CACHE = {}
LAST_RESULT = None


def _build():
    ablate = set(os.environ.get("KERNEL_ABLATE", "").split(","))
    nc = bacc.Bacc("TRN2", target_bir_lowering=False, debug=False, num_devices=8)

    xT_d = nc.dram_tensor("xT", [C, T], F32R, kind="ExternalInput").ap()
    wq_d = nc.dram_tensor("wq", [C, HC], F32R, kind="ExternalInput").ap()
    wk_d = nc.dram_tensor("wk", [C, HC], F32R, kind="ExternalInput").ap()
    wv_d = nc.dram_tensor("wv", [C, HC], F32R, kind="ExternalInput").ap()
    bq_d = nc.dram_tensor("bq", [HC], F32, kind="ExternalInput").ap()
    bk_d = nc.dram_tensor("bk", [HC], F32, kind="ExternalInput").ap()
    wo_d = nc.dram_tensor("wo", [HC, C], F32R, kind="ExternalInput").ap()
    y_d = nc.dram_tensor("y", [T, C], F32, kind="ExternalOutput").ap()

    with tile.TileContext(nc) as tc:
        with (
            tc.tile_pool(name="const", bufs=1) as const,
            tc.tile_pool(name="big", bufs=1) as big,
            tc.tile_pool(name="ysb", bufs=3) as ysb,
            tc.tile_pool(name="stats", bufs=24) as stats,
            tc.tile_pool(name="ps_s", bufs=4, space="PSUM") as ps_s,
            tc.tile_pool(name="ps_t", bufs=2, space="PSUM") as ps_t,
            tc.tile_pool(name="ps_o", bufs=2, space="PSUM") as ps_o,
        ):
            ps_proj = ps_t  # proj/outproj psum shares the transpose slots (tag)
            # ---- constants / inputs in SBUF ----
            ins_pool = tc.tile_pool(name="ins", bufs=1)
            ins = ins_pool.__enter__()
            xT = ins.tile([P, CG, T], F32R)
            xTr = xT_d.rearrange("(o p) t -> p o t", p=P)
            for tg in range(NG):
                nc.sync.dma_start(xT[:, :, ts(tg, 512)], xTr[:, :, ts(tg, 512)])
            wq = ins.tile([P, CG, HC], F32R)
            nc.sync.dma_start(wq, wq_d.rearrange("(o p) n -> p o n", p=P))
            wk = ins.tile([P, CG, HC], F32R)
            nc.sync.dma_start(wk, wk_d.rearrange("(o p) n -> p o n", p=P))
            wv = ins.tile([P, CG, HC], F32R)
            nc.sync.dma_start(wv, wv_d.rearrange("(o p) n -> p o n", p=P))
            wo = const.tile([P, HPAIRS, C], F32R)
            nc.sync.dma_start(wo, wo_d.rearrange("(o p) n -> p o n", p=P))
            bq = const.tile([P, HPAIRS], F32)
            nc.sync.dma_start(bq, bq_d.rearrange("(o p) -> p o", p=P))
            bk = const.tile([P, HPAIRS], F32)
            nc.sync.dma_start(bk, bk_d.rearrange("(o p) -> p o", p=P))

            ident = const.tile([P, P], BF16)
            make_identity(nc, ident)
            # cmask[:, :128] lower-triangular 0/-1e30, cmask[:, 128:256] all -1e30
            cmask = const.tile([P, 256], BF16)
            nc.gpsimd.memset(cmask, 0.0)
            nc.gpsimd.affine_select(
                out=cmask,
                in_=cmask,
                compare_op=OP.is_ge,
                fill=NEG,
                base=0,
                pattern=[[-1, 256]],
                channel_multiplier=1,
            )

            # ---- persistent intermediates ----
            QT = big.tile([P, HPAIRS, T], F32R)   # rows: head-pair's 2 heads x 64, scaled by 8, +bias
            KT = big.tile([P, HPAIRS, T], F32R)
            VS = big.tile([P, TT, HC], BF16)      # V rows: tokens, cols: 4 heads x 64
            OT = big.tile([P, HPAIRS, T], F32R)   # context^T rows: channels
            if "pv" in ablate or "attn" in ablate:
                nc.vector.memset(OT, 0.0)

            # ---- QKV projections (V interleaved with hp0 so attention starts early) ----
            for hp in range(HPAIRS):
                for tg in range(NG):
                    q_ps = ps_proj.tile([P, 512], F32, tag="pT")
                    for c in range(CG):
                        nc.tensor.matmul(
                            q_ps,
                            wq[:, c, ts(hp, P)],
                            xT[:, c, ts(tg, 512)],
                            start=(c == 0),
                            stop=(c == CG - 1),
                        )
                    # QT = (psum + bq) * 8   (fold sqrt(D) score scale into Q)
                    nc.vector.tensor_scalar(
                        QT[:, hp, ts(tg, 512)], q_ps, bq[:, hp : hp + 1], 8.0,
                        OP.add, OP.mult,
                    )
                    k_ps = ps_proj.tile([P, 512], F32, tag="pT")
                    for c in range(CG):
                        nc.tensor.matmul(
                            k_ps,
                            wk[:, c, ts(hp, P)],
                            xT[:, c, ts(tg, 512)],
                            start=(c == 0),
                            stop=(c == CG - 1),
                        )
                    nc.vector.tensor_scalar(
                        KT[:, hp, ts(tg, 512)], k_ps, bk[:, hp : hp + 1], None,
                        OP.add,
                    )
                    if hp == 0:
                        for tt in range(4 * tg, 4 * tg + 4):
                            v_ps = ps_proj.tile([P, HC], F32, tag="pT")
                            for c in range(CG):
                                nc.tensor.matmul(
                                    v_ps,
                                    xT[:, c, ts(tt, P)],
                                    wv[:, c, :],
                                    start=(c == 0),
                                    stop=(c == CG - 1),
                                )
                            nc.scalar.copy(VS[:, tt, :], v_ps)
            ins_pool.__exit__(None, None, None)
            pp_pool = tc.tile_pool(name="pp", bufs=12)
            pp = pp_pool.__enter__()
            pts_pool = tc.tile_pool(name="pts", bufs=6)
            pts = pts_pool.__enter__()

            # ---- attention per (head, q-group) ----
            for hp in range(HPAIRS if "attn" not in ablate else 0):
                for h in range(2):
                    hrow = 64 * h
                    hcol = (2 * hp + h) * 64
                    for g in range(NG):
                        # ---- phase A: online softmax over 512-wide parts ----
                        p_tiles = {}
                        for qc in range(4):
                            qt = 4 * g + qc
                            wl = WLAST[qc]
                            L = 512 * g + wl
                            np_ = g + 1
                            p_t = pp.tile([P, T], BF16, tag="P")
                            mparts = stats.tile([P, 4], F32, tag="mp")
                            sparts = stats.tile([P, 4], F32, tag="sp")
                            for i in range(np_):
                                w = 512 if i < g else wl
                                s_ps = ps_s.tile([P, 512], F32, tag="S")
                                nc.tensor.matmul(
                                    s_ps[:, :w],
                                    QT[hrow : hrow + 64, hp, ts(qt, P)],
                                    KT[hrow : hrow + 64, hp, ds(512 * i, w)],
                                    start=True,
                                    stop=True,
                                )
                                if i == g:
                                    # causal mask on the diagonal 128 (+128 pad for qc=0)
                                    mw = 256 if qc == 0 else 128
                                    dof = 128 * qc
                                    nc.vector.tensor_add(
                                        s_ps[:, dof : dof + mw],
                                        s_ps[:, dof : dof + mw],
                                        cmask[:, :mw],
                                    )
                                # negated per-part row max -> exp bias directly
                                nc.vector.reduce_max(
                                    mparts[:, i : i + 1], s_ps[:, :w],
                                    axis=AX.X, negate=True,
                                )
                                nc.scalar.activation(
                                    p_t[:, ds(512 * i, w)], s_ps[:, :w], ACTF.Exp,
                                    bias=mparts[:, i : i + 1], scale=1.0,
                                    accum_out=sparts[:, i : i + 1],
                                )
                            p_tiles[qc] = (p_t, L)
                            # batched combine: m = max_i m_i; f_i = exp(m_i - m)/s
                            negm = stats.tile([P, 1], F32, tag="negm")
                            if np_ == 1:
                                nc.vector.reciprocal(negm, sparts[:, :1])
                                nc.gpsimd.tensor_scalar(
                                    p_t[:, :L], p_t[:, :L], negm, None, OP.mult
                                )
                            else:
                                nc.vector.tensor_reduce(
                                    negm, mparts[:, :np_], axis=AX.X, op=OP.min
                                )
                                e = stats.tile([P, 4], F32, tag="e")
                                nc.scalar.activation(
                                    e[:, :np_], mparts[:, :np_], ACTF.Exp,
                                    bias=negm, scale=-1.0,
                                )
                                z = stats.tile([P, 4], F32, tag="z")
                                nc.vector.tensor_tensor(
                                    z[:, :np_], sparts[:, :np_], e[:, :np_], OP.mult
                                )
                                s = stats.tile([P, 1], F32, tag="s")
                                nc.vector.reduce_sum(s, z[:, :np_], axis=AX.X)
                                r = stats.tile([P, 1], F32, tag="r")
                                nc.vector.reciprocal(r, s)
                                f = stats.tile([P, 4], F32, tag="f")
                                nc.vector.tensor_scalar(
                                    f[:, :np_], e[:, :np_], r, None, OP.mult
                                )
                                for i in range(np_):
                                    w = 512 if i < g else wl
                                    nc.gpsimd.tensor_scalar(
                                        p_t[:, ds(512 * i, w)], p_t[:, ds(512 * i, w)],
                                        f[:, i : i + 1], None, OP.mult,
                                    )

                        # ---- phase B: transpose P blocks, P^T @ V ----
                        if "pv" in ablate:
                            continue
                        o_ps = ps_o.tile([64, 512], F32, tag="O")
                        nks = 4 * g + 4
                        for ks in range(nks):
                            lsd = ks - 4 * g
                            if lsd < 2:
                                qstart = 0
                            elif lsd == 2:
                                qstart = 2
                            else:
                                qstart = 3
                            pt_ps = ps_t.tile([P, 512], BF16, tag="pT")
                            for qc in range(qstart, 4):
                                p_t, _L = p_tiles[qc]
                                nc.tensor.matmul(
                                    pt_ps[:, ts(qc, P)],
                                    p_t[:, ts(ks, P)],
                                    ident,
                                    is_transpose=True,
                                    skip_group_check=True,
                                )
                            pt_sb = pts.tile([P, 512], BF16, tag="pTs")
                            if ks % 3 != 2:
                                nc.vector.tensor_copy(
                                    pt_sb[:, qstart * P :], pt_ps[:, qstart * P :]
                                )
                            else:
                                nc.scalar.copy(
                                    pt_sb[:, qstart * P :], pt_ps[:, qstart * P :]
                                )
                            nc.tensor.matmul(
                                o_ps[:, qstart * P :],
                                VS[:, ks, hcol : hcol + 64],
                                pt_sb[:, qstart * P :],
                                start=(ks == 0),
                                stop=(ks == nks - 1),
                                skip_group_check=True,
                            )
                        nc.vector.tensor_copy(
                            OT[hrow : hrow + 64, hp, ts(g, 512)], o_ps
                        )

                        # ---- output projection for this token group once all
                        # heads' context is ready (overlaps later attention) ----
                        if hp == 1 and h == 1:
                            for tt in range(4 * g, 4 * g + 4):
                                for n in range(2):
                                    y_ps = ps_o.tile([P, 512], F32, tag="O")
                                    for hpp in range(HPAIRS):
                                        nc.tensor.matmul(
                                            y_ps,
                                            OT[:, hpp, ts(tt, P)],
                                            wo[:, hpp, ts(n, 512)],
                                            start=(hpp == 0),
                                            stop=(hpp == HPAIRS - 1),
                                        )
                                    y_sb = ysb.tile([P, 512], F32, tag="y")
                                    nc.scalar.copy(y_sb, y_ps)
                                    nc.sync.dma_start(
                                        y_d[ts(tt, P), ts(n, 512)], y_sb
                                    )

            pts_pool.__exit__(None, None, None)
            pp_pool.__exit__(None, None, None)

    nc.compile()
    return nc


def kernel(x, w_qkv, b_qkv, b_out, w_out=None, **kw):
    # tolerate arbitrary kwarg order; reference signature is
    # (x, w_qkv, b_qkv, w_out, b_out)
    if w_out is None:
        w_out = kw.pop("w_out")
    global LAST_RESULT
    x = np.asarray(x, dtype=np.float32)
    w_qkv = np.asarray(w_qkv, dtype=np.float32)
    b_qkv = np.asarray(b_qkv, dtype=np.float32)
    w_out = np.asarray(w_out, dtype=np.float32)
    b_out = np.asarray(b_out, dtype=np.float32)

    if "nc" not in _CACHE:
        _CACHE["nc"] = _build()
    nc = _CACHE["nc"]

    xTs = [np.ascontiguousarray(x[b].T) for b in range(B)]
    in_maps = []
    for c in range(8):
        b = c // 4
        k4 = c % 4
        cols = slice(HC * k4, HC * k4 + HC)
        in_maps.append(
            {
                "xT": xTs[b],
                "wq": np.ascontiguousarray(w_qkv[:, cols]),
                "wk": np.ascontiguousarray(w_qkv[:, C + cols.start : C + cols.stop]),
                "wv": np.ascontiguousarray(
                    w_qkv[:, 2 * C + cols.start : 2 * C + cols.stop]
                ),
                "bq": np.ascontiguousarray(b_qkv[cols]),
                "bk": np.ascontiguousarray(b_qkv[C + cols.start : C + cols.stop]),
                "wo": np.ascontiguousarray(w_out[cols, :]),
            }
        )

    res = run_bass_kernel_spmd(nc, in_maps, core_ids=list(range(8)))
    LAST_RESULT = res

    y = np.zeros((B, T, C), dtype=np.float32)
    for c in range(8):
        y[c // 4] += res.results[c]["y"]
    # constant terms: V-bias flows through softmax (weights sum to 1) as a
    # constant row shift, so its contribution is exactly b_v @ w_out; plus b_out.
    b_v = b_qkv[2 * C :]
    y += (b_v @ w_out + b_out).astype(np.float32)
    return y

